# revision 16
# baseline (speedup 1.0000x reference)
# Distributed kNN-retrieval loss kernel for Trainium2 (8 NeuronCores), v2.
#
# Reference computation: two linear heads + softmax, a feature bank updated at
# trg_idx rows (no-grad), cosine kNN against the bank, KL pseudo-label loss +
# entropy/IM + label-smoothed CE. Output: scalar loss.
#
# Strategy vs v1 (214 us):
#  * fea_bank shipped pre-transposed [D, N/8] in fp8 e4m3, scaled x16 (unit
#    rows ~N(0, 1/sqrt D) sit in e4m3's subnormal range unscaled). Positive
#    scaling never changes a row's top-k order. The big matmul runs in
#    DoubleRow perf mode (2 K-tiles per instruction, 0.5 cyc/row): ~2x PE and
#    2x less HBM than v1's bf16.
#  * Per 2500-col window: PSUM->SBUF copies (ACT) emit a bf16 dist canvas;
#    DVE max8 + find_index8 run at the 16-bit rate. Window top-8
#    values/indices accumulate into per-core candidate canvases.
#  * Candidate exchange: each core reduces its 80 window-candidates to its
#    local top-8 (exact lexicographic (bf16 value, column) packing:
#    p = v*4096 + 2^21 + col is exact in f32 for |v|<4096, so value-matching
#    is collision-free by construction), then ONE AllGather of [256,16]
#    replaces v1's 6 pipelined collectives + 24 strided tail DMAs.
#  * Bank update handled without touching the bank: G = fn@fn.T (f32 PE,
#    scaled into the x16 raw-dist scale, bf16-rounded so all merge values
#    share one grid) supplies the 8 post-update trg-column candidates; the
#    stale streamed trg columns are left in place (~2e-4 loss risk, as v1).
#  * Merge to global top-6 on every core identically: pack -> max8 -> 5
#    match-accum index recoveries, drop slot 0 (self). Winner scores come
#    from one indirect-DMA gather path against the p_aad-patched score bank;
#    kl/entropy/IM/CE reduce on device; host reads core 0's scalar.
#  * All heavy DMAs dispatch from the Pool queue (25 ns vs 565 ns on sync);
#    head/G/stat sections are emitted interleaved with the stream so the
#    serial prologue/tail of v1 (~126 us combined) collapses.

import ml_dtypes
import numpy as np

import concourse.bass as bass
import concourse.mybir as mybir
import concourse.tile as tile
from concourse import bacc
from concourse.bass import IndirectOffsetOnAxis
from concourse.bass_utils import run_bass_kernel_spmd

F32 = mybir.dt.float32
BF16 = mybir.dt.bfloat16
FP8 = mybir.dt.float8e4
U16 = mybir.dt.uint16
U32 = mybir.dt.uint32
AF = mybir.ActivationFunctionType
ALU = mybir.AluOpType
AX = mybir.AxisListType
DR = mybir.MatmulPerfMode.DoubleRow

# Problem sizes (hardcoded per harness contract)
B = 256          # batch
D = 512          # feature dim
C = 10           # classes
N = 200000       # bank rows
K = 5            # neighbors
EPS_LS = 0.1
ENT_WT, IM_WT, AAD_WT, TGT_WT = 1.0, 1.0, 1.0, 0.1

P = 128          # partitions
NM = B // P      # row tiles (2)
KD = D // P      # contraction slices (4)
KP = KD // 2     # DoubleRow k-pairs (2)

NCORES = 8
NLOC = N // NCORES        # 25000
TN = 500                  # matmul free-dim tile (1 PSUM bank of f32)
NTW = 5                   # n-tiles per window
WWIN = TN * NTW           # 2500
NWIN = NLOC // WWIN       # 10
NCAND = NWIN * 8          # window candidates per core (80)
NMERGE = NCORES * 8 + 6   # merged candidates (70)

SCALE = 16.0              # fp8 bank pre-scale
PACK = 4096.0             # (bf16 value, col) lexicographic packing radix
PBIAS = float(2 ** 21)    # packing bias keeping packed values positive
BIGNEG = -1.0e30


def build_program(ncores=NCORES, nloc=NLOC, nwin=NWIN, debug=False):
    """Builds the SPMD Bass program (same program on every core)."""
    n_total = ncores * nloc

    nc = bacc.Bacc(
        "TRN2", target_bir_lowering=False, debug=False, num_devices=ncores
    )

    # ---- I/O ----
    fb8_h = nc.dram_tensor("fb8", [D, nloc], FP8, kind="ExternalInput")
    tf8_h = nc.dram_tensor("tf8", [P, KD, B], FP8, kind="ExternalInput")
    tfT_h = nc.dram_tensor("tfT", [D, B], F32, kind="ExternalInput")
    cb_h = nc.dram_tensor("core_base", [P, 1], F32, kind="ExternalInput")
    gmask_h = nc.dram_tensor("gmask", [1, B], F32, kind="ExternalInput")
    Wm_h = nc.dram_tensor("Wm", [D, C], F32, kind="ExternalInput")
    bm_h = nc.dram_tensor("bm", [1, C], F32, kind="ExternalInput")
    Wa_h = nc.dram_tensor("Wa", [D, C], F32, kind="ExternalInput")
    ba_h = nc.dram_tensor("ba", [1, C], F32, kind="ExternalInput")
    sb_h = nc.dram_tensor("sbank", [n_total, C], F32, kind="ExternalInput")
    c80s_h = nc.dram_tensor("c80s", [P, 8], F32, kind="ExternalInput")
    bc256_h = nc.dram_tensor("bc256", [P, B], F32, kind="ExternalInput")
    tidxu_h = nc.dram_tensor("tidxu", [B, 1], U32, kind="ExternalInput")
    tidxf_h = nc.dram_tensor("tidxf", [1, B], F32, kind="ExternalInput")
    loss_h = nc.dram_tensor("loss", [1, 1], F32, kind="ExternalOutput")

    def dump(name, ap):
        if not debug:
            return
        t = nc.dram_tensor(f"dbg_{name}", list(ap.shape), ap.dtype,
                           kind="ExternalOutput")
        nc.sync.dma_start(t.ap()[tuple(slice(0, d) for d in ap.shape)], ap)

    with tile.TileContext(nc) as tc:
        with (
            tc.tile_pool(name="const", bufs=1) as cp,
            tc.tile_pool(name="fb", bufs=6) as fp,
            tc.tile_pool(name="dist", bufs=2) as dp,
            tc.tile_pool(name="scratch", bufs=2) as sp,
            tc.tile_pool(name="psA", bufs=7, space="PSUM") as ppA,
            tc.tile_pool(name="psB", bufs=1, space="PSUM") as ppB,
            tc.tile_pool(name="dram", bufs=1, space="DRAM") as dr,
        ):
            # ---------- window DMA / compute emitters ----------
            fb8_r = fb8_h.ap().rearrange("(kp j p) c -> kp p j c", kp=KP, j=2)

            def emit_window_dma(w):
                tiles = [fp.tile([P, 2, WWIN], FP8, tag=f"fb8_{kp}",
                                 name=f"fb8_{kp}") for kp in range(KP)]
                for kp in range(KP):
                    nc.gpsimd.dma_start(
                        tiles[kp][:],
                        fb8_r[kp, :, :, w * WWIN:(w + 1) * WWIN],
                    )
                return tiles

            def emit_window_compute(w, tiles):
                dbb = [dp.tile([P, WWIN], BF16, tag=f"dbb{m}", name=f"dbb{m}")
                       for m in range(NM)]
                for m in range(NM):
                    msl = slice(m * P, (m + 1) * P)
                    pst = []
                    for t in range(NTW):
                        ps = ppA.tile([P, TN], F32, tag="ps_big", name="ps_big")
                        pst.append(ps)
                    for kp in range(KP):
                        for t in range(NTW):
                            nc.tensor.matmul(
                                pst[t][:],
                                lhsT=tf8_sb[:, 2 * kp:2 * kp + 2, msl],
                                rhs=tiles[kp][:, :, t * TN:(t + 1) * TN],
                                start=(kp == 0), stop=(kp == KP - 1),
                                perf_mode=DR,
                            )
                    for t in range(NTW):
                        nc.scalar.copy(dbb[m][:, t * TN:(t + 1) * TN], pst[t][:])
                for m in range(NM):
                    v8 = sp.tile([P, 8], BF16, tag="v8", name="v8")
                    nc.vector.max(out=v8[:], in_=dbb[m][:])
                    i8 = sp.tile([P, 8], U16, tag="i8", name="i8")
                    nc.vector.max_index(i8[:], v8[:], dbb[m][:])
                    # bookkeeping on Pool: packed value + global index slices
                    csl = slice(w * 8, (w + 1) * 8)
                    nc.gpsimd.tensor_scalar(
                        candP[m][:, csl], v8[:], PACK, float(PBIAS + w * 8),
                        ALU.mult, ALU.add,
                    )
                    nc.gpsimd.tensor_add(
                        candP[m][:, csl], candP[m][:, csl], c80s_sb[:]
                    )
                    nc.gpsimd.tensor_copy(candI[m][:, csl], i8[:])
                    nc.gpsimd.tensor_scalar(
                        candI[m][:, csl], candI[m][:, csl], cb_sb[:],
                        float(w * WWIN), ALU.add, ALU.add,
                    )

            # ---------- first window DMAs (Pool queue head) ----------
            win_tiles = {}
            win_tiles[0] = emit_window_dma(0)
            win_tiles[1] = emit_window_dma(1)

            # ---------- constants into SBUF (sync queue, parallel) ----------
            tf8_sb = cp.tile([P, KD, B], FP8, tag="tf8", name="tf8")
            nc.sync.dma_start(tf8_sb[:], tf8_h.ap()[:, :, :])
            tfT = [cp.tile([P, B], F32, tag=f"tfT{k}", name=f"tfT{k}")
                   for k in range(KD)]
            for k in range(KD):
                nc.sync.dma_start(tfT[k][:], tfT_h.ap()[k * P:(k + 1) * P, :])
            cb_sb = cp.tile([P, 1], F32, tag="cb", name="cb")
            nc.sync.dma_start(cb_sb[:], cb_h.ap()[:, :])
            gmask_sb = cp.tile([1, B], F32, tag="gmask", name="gmask")
            nc.sync.dma_start(gmask_sb[:], gmask_h.ap()[:, :])
            c80s_sb = cp.tile([P, 8], F32, tag="c80s", name="c80s")
            nc.sync.dma_start(c80s_sb[:], c80s_h.ap()[:, :])
            bc256_sb = cp.tile([P, B], F32, tag="bc256", name="bc256")
            nc.sync.dma_start(bc256_sb[:], bc256_h.ap()[:, :])
            tidxu_sb = [cp.tile([P, 1], U32, tag=f"tidxu{m}", name=f"tidxu{m}")
                        for m in range(NM)]
            for m in range(NM):
                nc.sync.dma_start(tidxu_sb[m][:], tidxu_h.ap()[m * P:(m + 1) * P, :])
            tidxf_sb = cp.tile([1, B], F32, tag="tidxf", name="tidxf")
            nc.sync.dma_start(tidxf_sb[:], tidxf_h.ap()[:, :])

            Wsb = {}
            bsb = {}
            for name, Wh, bh in (("m", Wm_h, bm_h), ("a", Wa_h, ba_h)):
                Wsb[name] = [cp.tile([P, C], F32, tag=f"W{name}{k}",
                                     name=f"W{name}{k}") for k in range(KD)]
                for k in range(KD):
                    nc.sync.dma_start(Wsb[name][k][:], Wh.ap()[k * P:(k + 1) * P, :])
                bsb[name] = cp.tile([1, C], F32, tag=f"b{name}", name=f"b{name}")
                nc.sync.dma_start(bsb[name][:], bh.ap()[:, :])

            ones_k1 = cp.tile([1, P], F32, tag="ones_k1", name="ones_k1")
            nc.vector.memset(ones_k1[:], 1.0)
            ones_m1 = cp.tile([P, 1], F32, tag="ones_m1", name="ones_m1")
            nc.vector.memset(ones_m1[:], 1.0)
            eps_b = cp.tile([P, 1], F32, tag="eps_b", name="eps_b")
            nc.vector.memset(eps_b[:], 1e-5)

            # candidate canvases (packed values + global indices)
            candP = [sp.tile([P, NCAND], F32, tag=f"candP{m}", name=f"candP{m}",
                             bufs=1) for m in range(NM)]
            candI = [sp.tile([P, NCAND], F32, tag=f"candI{m}", name=f"candI{m}",
                             bufs=1) for m in range(NM)]

            # ---------- stream windows 0-1 ----------
            emit_window_compute(0, win_tiles[0])
            win_tiles[2] = emit_window_dma(2)
            emit_window_compute(1, win_tiles[1])
            win_tiles[3] = emit_window_dma(3)

            # ---------- classifier heads + softmax ----------
            p_t = {"m": [], "a": []}
            pmax_t = {"m": [], "a": []}
            logp_t = []
            for m in range(NM):
                msl = slice(m * P, (m + 1) * P)
                for name in ("m", "a"):
                    ps = ppB.tile([P, C], F32, tag="psB", name="ps_small")
                    for k in range(KD):
                        nc.tensor.matmul(
                            ps[:], lhsT=tfT[k][:, msl], rhs=Wsb[name][k][:],
                            start=(k == 0), stop=False,
                        )
                    nc.tensor.matmul(
                        ps[:], lhsT=ones_k1[:], rhs=bsb[name][:],
                        start=False, stop=True,
                    )
                    lg = cp.tile([P, C], F32, tag=f"lg{name}{m}", name=f"lg{name}{m}")
                    nc.scalar.copy(lg[:], ps[:])
                    mx = cp.tile([P, 1], F32, tag=f"mx{name}{m}", name=f"mx{name}{m}")
                    nc.vector.reduce_max(mx[:], lg[:], axis=AX.X)
                    negmx = sp.tile([P, 1], F32, tag="negmx", name="negmx")
                    nc.vector.tensor_scalar_mul(negmx[:], mx[:], -1.0)
                    exps = sp.tile([P, C], F32, tag="exps", name="exps")
                    sumexp = cp.tile([P, 1], F32, tag=f"se{name}{m}", name=f"se{name}{m}")
                    nc.scalar.activation(
                        exps[:], lg[:], AF.Exp, bias=negmx[:], scale=1.0,
                        accum_out=sumexp[:],
                    )
                    rcp = sp.tile([P, 1], F32, tag="rcp", name="rcp")
                    nc.vector.reciprocal(rcp[:], sumexp[:])
                    pp = cp.tile([P, C], F32, tag=f"p{name}{m}", name=f"p{name}{m}")
                    nc.vector.tensor_scalar_mul(pp[:], exps[:], rcp[:])
                    p_t[name].append(pp)
                    pm = cp.tile([P, 1], F32, tag=f"pmax{name}{m}", name=f"pmax{name}{m}")
                    nc.vector.reduce_max(pm[:], pp[:], axis=AX.X)
                    pmax_t[name].append(pm)
                    if name == "m":
                        lnS = sp.tile([P, 1], F32, tag="lnS", name="lnS")
                        nc.scalar.activation(lnS[:], sumexp[:], AF.Ln)
                        logZ = sp.tile([P, 1], F32, tag="logZ", name="logZ")
                        nc.vector.tensor_add(logZ[:], lnS[:], mx[:])
                        lp = cp.tile([P, C], F32, tag=f"logp{m}", name=f"logp{m}")
                        nc.vector.tensor_scalar_sub(lp[:], lg[:], logZ[:])
                        logp_t.append(lp)
                    dump(f"p_{name}{m}", pp[:])

            # patch this core's score_bank copy in place: rows trg_idx <- p_aad
            for m in range(NM):
                nc.gpsimd.indirect_dma_start(
                    out=sb_h.ap(),
                    out_offset=IndirectOffsetOnAxis(ap=tidxu_sb[m][:], axis=0),
                    in_=p_t["a"][m][:],
                    in_offset=None,
                )

            # ---------- stream windows 2-3 ----------
            emit_window_compute(2, win_tiles[2])
            win_tiles[4] = emit_window_dma(4)
            emit_window_compute(3, win_tiles[3])
            win_tiles[5] = emit_window_dma(5)

            # ---------- G = scaled post-update trg-column sims ----------
            ps2 = ppB.tile([1, B], F32, tag="psB", name="ps_s2")
            for k in range(KD):
                sq = sp.tile([P, B], F32, tag="sq", name="sq")
                nc.scalar.square(sq[:], tfT[k][:])
                nc.tensor.matmul(
                    ps2[:], lhsT=ones_m1[:], rhs=sq[:],
                    start=(k == 0), stop=(k == KD - 1),
                )
            srow = cp.tile([1, B], F32, tag="srow", name="srow")
            nc.scalar.sqrt(srow[:], ps2[:])
            invs = cp.tile([1, B], F32, tag="invs", name="invs")
            nc.vector.reciprocal(invs[:], srow[:])
            nc.vector.tensor_scalar_mul(invs[:], invs[:], SCALE)
            invs_bc = cp.tile([P, B], F32, tag="invs_bc", name="invs_bc")
            nc.gpsimd.partition_broadcast(invs_bc[:], invs[:])
            gm_bc = cp.tile([P, B], F32, tag="gm_bc", name="gm_bc")
            nc.gpsimd.partition_broadcast(gm_bc[:], gmask_sb[:])
            tidx_bc = cp.tile([P, B], F32, tag="tidx_bc", name="tidx_bc")
            nc.gpsimd.partition_broadcast(tidx_bc[:], tidxf_sb[:])

            Gpk8 = []
            GidxF = []
            for m in range(NM):
                msl = slice(m * P, (m + 1) * P)
                psG = ppB.tile([P, B], F32, tag="psB", name="ps_G")
                for k in range(KD):
                    nc.tensor.matmul(
                        psG[:], lhsT=tfT[k][:, msl], rhs=tfT[k][:],
                        start=(k == 0), stop=(k == KD - 1),
                    )
                Gp = sp.tile([P, B], F32, tag="Gp", name="Gp")
                nc.vector.tensor_mul(Gp[:], psG[:], invs_bc[:])
                nc.vector.tensor_add(Gp[:], Gp[:], gm_bc[:])
                Gbf = sp.tile([P, B], BF16, tag="Gbf", name="Gbf")
                nc.scalar.copy(Gbf[:], Gp[:])
                Gpk = sp.tile([P, B], F32, tag="Gpk", name="Gpk")
                nc.vector.scalar_tensor_tensor(
                    Gpk[:], in0=Gbf[:], scalar=PACK, in1=bc256_sb[:],
                    op0=ALU.mult, op1=ALU.add,
                )
                g8p = cp.tile([P, 8], F32, tag=f"g8p{m}", name=f"g8p{m}")
                nc.vector.max(out=g8p[:], in_=Gpk[:])
                gf = cp.tile([P, 6], F32, tag=f"GidxF{m}", name=f"GidxF{m}")
                for s in range(6):
                    gtmp = sp.tile([P, B], F32, tag="gtmp", name="gtmp")
                    nc.vector.scalar_tensor_tensor(
                        gtmp[:], in0=Gpk[:], scalar=g8p[:, s:s + 1],
                        in1=tidx_bc[:], op0=ALU.is_equal, op1=ALU.mult,
                        accum_out=gf[:, s:s + 1],
                    )
                Gpk8.append(g8p)
                GidxF.append(gf)
                dump(f"gpk8_{m}", g8p[:])
                dump(f"gidx{m}", gf[:])

            # ---------- stream windows 4-5 ----------
            emit_window_compute(4, win_tiles[4])
            win_tiles[6] = emit_window_dma(6)
            emit_window_compute(5, win_tiles[5])
            win_tiles[7] = emit_window_dma(7)

            # ---------- entropy / IM / CE terms ----------
            stat = [cp.tile([P, 3], F32, tag=f"stat{m}", name=f"stat{m}")
                    for m in range(NM)]
            for m in range(NM):
                lp5 = sp.tile([P, C], F32, tag="lp5", name="lp5")
                nc.scalar.activation(lp5[:], p_t["m"][m][:], AF.Ln, bias=eps_b[:])
                pe = sp.tile([P, C], F32, tag="pe", name="pe")
                entneg = sp.tile([P, 1], F32, tag="entneg", name="entneg")
                nc.vector.scalar_tensor_tensor(
                    pe[:], in0=p_t["m"][m][:], scalar=1.0, in1=lp5[:],
                    op0=ALU.mult, op1=ALU.mult, accum_out=entneg[:],
                )
                nc.vector.tensor_copy(stat[m][:, 1:2], entneg[:])

                pickm = sp.tile([P, 1], U32, tag="pickm", name="pickm")
                nc.vector.tensor_tensor(
                    pickm[:], pmax_t["m"][m][:], pmax_t["a"][m][:], op=ALU.is_gt
                )
                chosen = sp.tile([P, C], F32, tag="chosen", name="chosen")
                nc.vector.select(
                    chosen[:], pickm[:].to_broadcast([P, C]),
                    p_t["m"][m][:], p_t["a"][m][:],
                )
                c8 = sp.tile([P, 8], F32, tag="c8", name="c8")
                nc.vector.max(out=c8[:], in_=chosen[:])
                ohlp = sp.tile([P, C], F32, tag="ohlp", name="ohlp")
                lpsel = sp.tile([P, 1], F32, tag="lpsel", name="lpsel")
                nc.vector.scalar_tensor_tensor(
                    ohlp[:], in0=chosen[:], scalar=c8[:, 0:1],
                    in1=logp_t[m][:], op0=ALU.is_equal, op1=ALU.mult,
                    accum_out=lpsel[:],
                )
                slogp = sp.tile([P, 1], F32, tag="slogp", name="slogp")
                nc.vector.reduce_sum(slogp[:], logp_t[m][:], axis=AX.X)
                sl001 = sp.tile([P, 1], F32, tag="sl001", name="sl001")
                nc.vector.tensor_scalar_mul(sl001[:], slogp[:], EPS_LS / C)
                cerow = sp.tile([P, 1], F32, tag="cerow", name="cerow")
                nc.vector.scalar_tensor_tensor(
                    cerow[:], in0=lpsel[:], scalar=(1.0 - EPS_LS), in1=sl001[:],
                    op0=ALU.mult, op1=ALU.add,
                )
                nc.vector.tensor_scalar_mul(cerow[:], cerow[:], -1.0)
                nc.vector.tensor_copy(stat[m][:, 2:3], cerow[:])

            ps_mp = ppB.tile([1, C], F32, tag="psB", name="ps_mp")
            for m in range(NM):
                nc.tensor.matmul(
                    ps_mp[:], lhsT=ones_m1[:], rhs=p_t["m"][m][:],
                    start=(m == 0), stop=(m == NM - 1),
                )
            mp = cp.tile([1, C], F32, tag="mp", name="mp")
            nc.scalar.mul(mp[:], ps_mp[:], 1.0 / B)
            mplog = sp.tile([1, C], F32, tag="mplog", name="mplog")
            nc.scalar.activation(mplog[:], mp[:], AF.Ln, bias=eps_b[0:1, :])
            mpe = sp.tile([1, C], F32, tag="mpe", name="mpe")
            imsum = cp.tile([1, 1], F32, tag="imsum", name="imsum")
            nc.vector.scalar_tensor_tensor(
                mpe[:], in0=mp[:], scalar=1.0, in1=mplog[:],
                op0=ALU.mult, op1=ALU.mult, accum_out=imsum[:],
            )

            # ---------- stream windows 6-9 ----------
            emit_window_compute(6, win_tiles[6])
            win_tiles[8] = emit_window_dma(8)
            emit_window_compute(7, win_tiles[7])
            win_tiles[9] = emit_window_dma(9)
            emit_window_compute(8, win_tiles[8])
            emit_window_compute(9, win_tiles[9])

            # ---------- local reduce to core top-8 + exchange ----------
            cand_dr = dr.tile([B, 16], F32, tag="cand_dr", name="cand_dr")
            bounce = dr.tile([NCORES * B, 16], F32, tag="bounce", name="bounce")
            for m in range(NM):
                l8 = sp.tile([P, 8], F32, tag=f"loc8p{m}", name=f"loc8p{m}",
                             bufs=1)
                nc.vector.max(out=l8[:], in_=candP[m][:])
                locI = sp.tile([P, 8], F32, tag=f"locI{m}", name=f"locI{m}",
                               bufs=1)
                for s in range(8):
                    ltmp = sp.tile([P, NCAND], F32, tag="ltmp", name="ltmp",
                                   bufs=3)
                    nc.vector.scalar_tensor_tensor(
                        ltmp[:], in0=candP[m][:], scalar=l8[:, s:s + 1],
                        in1=candI[m][:], op0=ALU.is_equal, op1=ALU.mult,
                        accum_out=locI[:, s:s + 1],
                    )
                nc.gpsimd.dma_start(cand_dr[m * P:(m + 1) * P, 0:8], l8[:])
                nc.gpsimd.dma_start(cand_dr[m * P:(m + 1) * P, 8:16], locI[:])
                dump(f"l8_{m}", l8[:])
                dump(f"locI{m}", locI[:])

            nc.gpsimd.collective_compute(
                "AllGather",
                ALU.bypass,
                replica_groups=[list(range(ncores))],
                ins=[cand_dr[:].opt()],
                outs=[bounce[:].opt()],
            )

            # ---------- merge to global top-6, drop the max (self) ----------
            sg = bounce[:].rearrange("(c q p) (h e) -> q p c h e",
                                     c=ncores, q=NM, p=P, h=2)
            scr_tiles = []
            for m in range(NM):
                allP = sp.tile([P, NMERGE], F32, tag=f"allP{m}", name=f"allP{m}",
                               bufs=1)
                allI = sp.tile([P, NMERGE], F32, tag=f"allI{m}", name=f"allI{m}",
                               bufs=1)
                nall = ncores * 8
                nc.gpsimd.dma_start(
                    allP[:, 0:nall].rearrange("p (c e) -> p c e", c=ncores),
                    sg[m, :, :, 0, :],
                )
                nc.gpsimd.dma_start(
                    allI[:, 0:nall].rearrange("p (c e) -> p c e", c=ncores),
                    sg[m, :, :, 1, :],
                )
                nc.gpsimd.tensor_copy(allP[:, nall:NMERGE], Gpk8[m][:, 0:6])
                nc.gpsimd.tensor_copy(allI[:, nall:NMERGE], GidxF[m][:])
                dump(f"allP{m}", allP[:])
                dump(f"allI{m}", allI[:])

                win8p = sp.tile([P, 8], F32, tag="win8p", name="win8p")
                nc.vector.max(out=win8p[:], in_=allP[:])
                winIdx = sp.tile([P, K], F32, tag="winIdx", name="winIdx")
                for s in range(1, K + 1):   # slots 1..5 (slot 0 = self)
                    tmp = sp.tile([P, NMERGE], F32, tag="tmpm", name="tmpm",
                                  bufs=3)
                    nc.vector.scalar_tensor_tensor(
                        tmp[:], in0=allP[:], scalar=win8p[:, s:s + 1],
                        in1=allI[:], op0=ALU.is_equal, op1=ALU.mult,
                        accum_out=winIdx[:, s - 1:s],
                    )
                dump(f"winIdx{m}", winIdx[:])

                bidxf = sp.tile([P, K], F32, tag="bidxf", name="bidxf")
                nc.gpsimd.tensor_scalar_min(bidxf[:], winIdx[:], float(n_total - 1))
                bidx_u = sp.tile([P, K], U32, tag="bidxu", name="bidxu")
                nc.gpsimd.tensor_copy(bidx_u[:], bidxf[:])

                scr = sp.tile([P, K * C], F32, tag=f"scr{m}", name=f"scr{m}",
                              bufs=1)
                scr_tiles.append(scr)
                for k in range(K):
                    nc.gpsimd.indirect_dma_start(
                        out=scr[:, k * C:(k + 1) * C],
                        out_offset=None,
                        in_=sb_h.ap(),
                        in_offset=IndirectOffsetOnAxis(ap=bidx_u[:, k:k + 1], axis=0),
                    )
                dump(f"scr{m}", scr[:])

            # kl = s * (ln s - p_aad)
            for m in range(NM):
                scr = scr_tiles[m]
                pa_bc = p_t["a"][m][:, None, :].to_broadcast([P, K, C])
                lns = sp.tile([P, K * C], F32, tag="lns", name="lns")
                nc.scalar.activation(lns[:], scr[:], AF.Ln)
                dd = sp.tile([P, K * C], F32, tag="dd", name="dd")
                nc.vector.tensor_sub(
                    dd[:].rearrange("p (k c) -> p k c", k=K),
                    lns[:].rearrange("p (k c) -> p k c", k=K), pa_bc
                )
                klrow = sp.tile([P, 1], F32, tag="klrow", name="klrow")
                nc.vector.scalar_tensor_tensor(
                    dd[:], in0=scr[:], scalar=1.0, in1=dd[:],
                    op0=ALU.mult, op1=ALU.mult, accum_out=klrow[:],
                )
                nc.vector.tensor_copy(stat[m][:, 0:1], klrow[:])
                dump(f"stat{m}", stat[m][:])

            # ---------- final scalar reductions ----------
            ps_st = ppB.tile([1, 3], F32, tag="psB", name="ps_st")
            for m in range(NM):
                nc.tensor.matmul(
                    ps_st[:], lhsT=ones_m1[:], rhs=stat[m][:],
                    start=(m == 0), stop=(m == NM - 1),
                )
            stats = sp.tile([1, 3], F32, tag="stats", name="stats")
            nc.scalar.copy(stats[:], ps_st[:])

            t1 = sp.tile([1, 1], F32, tag="t1", name="t1")
            nc.vector.scalar_tensor_tensor(
                t1[:], in0=stats[:, 1:2], scalar=(-ENT_WT / B), in1=imsum[:],
                op0=ALU.mult, op1=ALU.add,
            )
            t2 = sp.tile([1, 1], F32, tag="t2", name="t2")
            nc.vector.scalar_tensor_tensor(
                t2[:], in0=stats[:, 0:1], scalar=(AAD_WT / B), in1=t1[:],
                op0=ALU.mult, op1=ALU.add,
            )
            t3 = sp.tile([1, 1], F32, tag="t3", name="t3")
            nc.vector.scalar_tensor_tensor(
                t3[:], in0=stats[:, 2:3], scalar=(TGT_WT / B), in1=t2[:],
                op0=ALU.mult, op1=ALU.add,
            )
            nc.sync.dma_start(loss_h.ap()[:, :], t3[:])

    nc.compile()
    return nc


def make_in_maps(trg_feat, W, b, W_aad, b_aad, fea_bank, score_bank, trg_idx,
                 ncores=NCORES, nloc=NLOC):
    n_total = ncores * nloc
    trg_feat = np.ascontiguousarray(np.asarray(trg_feat, dtype=np.float32))
    fea_bank = np.asarray(fea_bank, dtype=np.float32)
    score_bank = np.ascontiguousarray(np.asarray(score_bank, dtype=np.float32))
    trg_idx = np.asarray(trg_idx).astype(np.int64)

    # last-writer wins for duplicate trg_idx (matches .at[].set order)
    gmask = np.zeros((B,), dtype=np.float32)
    seen = set()
    for j in range(B - 1, -1, -1):
        if int(trg_idx[j]) in seen:
            gmask[j] = BIGNEG
        else:
            seen.add(int(trg_idx[j]))

    tfT = np.ascontiguousarray(trg_feat.T)                          # [D, B]
    tf8 = np.ascontiguousarray(
        tfT.reshape(KD, P, B).transpose(1, 0, 2).astype(ml_dtypes.float8_e4m3)
    )                                                               # [P, KD, B]
    fb8 = np.ascontiguousarray(
        (fea_bank.T * SCALE).astype(ml_dtypes.float8_e4m3)
    )                                                               # [D, N]

    # globally-unique packing columns: stream candidates of core c get
    # cols c*80..c*80+79; the G candidates get cols 640..895.
    bc256 = np.broadcast_to(
        np.float32(PBIAS) + ncores * NCAND + np.arange(B, dtype=np.float32),
        (P, B)).copy()
    common = {
        "tf8": tf8,
        "tfT": tfT,
        "gmask": gmask.reshape(1, B),
        "Wm": np.ascontiguousarray(np.asarray(W, dtype=np.float32)),
        "bm": np.asarray(b, dtype=np.float32).reshape(1, C),
        "Wa": np.ascontiguousarray(np.asarray(W_aad, dtype=np.float32)),
        "ba": np.asarray(b_aad, dtype=np.float32).reshape(1, C),
        "sbank": score_bank,
        "bc256": bc256,
        "tidxu": trg_idx.astype(np.uint32).reshape(B, 1),
        "tidxf": trg_idx.astype(np.float32).reshape(1, B),
    }
    in_maps = []
    for c in range(ncores):
        sl = slice(c * nloc, (c + 1) * nloc)
        c80s = np.broadcast_to(
            np.float32(c * NCAND) + np.arange(8, dtype=np.float32),
            (P, 8)).copy()
        in_maps.append(dict(
            common,
            fb8=np.ascontiguousarray(fb8[:, sl]),
            core_base=np.full((P, 1), float(c * nloc), dtype=np.float32),
            c80s=c80s,
        ))
    return in_maps


_cached_nc = None
last_results = None


def kernel(trg_feat, W, b, W_aad, b_aad, fea_bank, score_bank, trg_idx):
    global _cached_nc, last_results
    if _cached_nc is None:
        _cached_nc = build_program()
    in_maps = make_in_maps(
        trg_feat, W, b, W_aad, b_aad, fea_bank, score_bank, trg_idx
    )
    last_results = run_bass_kernel_spmd(
        _cached_nc, in_maps, core_ids=list(range(NCORES))
    )
    loss = np.asarray(last_results.results[0]["loss"], dtype=np.float32)
    return loss.reshape(())
